# revision 1
# baseline (speedup 1.0000x reference)
"""Deformable-conv stack (8 layers) on 8 Trainium2 NeuronCores.

Strategy:
  - Layer 0 (1x1 deform conv, 512->256) computed on host (x and off0 are
    kernel inputs, so the sampled im2col and the 1x1 conv are host numpy).
  - Layers 1..7 (3x3 deform convs) on device, data-parallel over
    (sample, image-half): core 2s+h handles rows 32h..32h+31 of sample s.
  - All sampling indices / bilinear weights precomputed on host.
  - Device per layer: pack Q4 (4 corners interleaved, padded 78x78 image),
    ap_gather per 3-tap chunk, DVE multiply by broadcast bilinear weights +
    inner-4 reduce -> im2col slice, PE matmuls accumulate in PSUM,
    ACT relu+bias eviction, pair AllGather to rebuild the full image.
"""
import time as _time
import numpy as np
import ml_dtypes
from contextlib import ExitStack

import concourse.bass as bass
import concourse.mybir as mybir
import concourse.tile as tile
from concourse import bass_utils
from concourse import bacc

bf16 = ml_dtypes.bfloat16

H = W = 64
PAD = 8
HP = WP = H + 2 * PAD          # 80
NPIX_PAD = HP * WP             # 6400
Q4_BUILD = (HP - 2) * WP + (WP - 2) + 1   # max valid q00 + 1
NPIX = H * W
PXH = NPIX // 2                # 2048
K = 3
NCORES = 8
NTAPS = 9
CHUNK_TAPS = 3
NI_CHUNK = CHUNK_TAPS * PXH    # 6144 indices per gather


# ---------------- host-side index/weight precompute ----------------

def _tap_indices_weights(off_l, k, pad):
    KK = int(round(np.sqrt(off_l.shape[0] // 2)))
    kh, kw = divmod(k, KK)
    dy = off_l[2 * k]
    dx = off_l[2 * k + 1]
    yy = np.arange(H, dtype=np.float64)[:, None]
    xx = np.arange(W, dtype=np.float64)[None, :]
    py = yy + (kh - pad) + dy.astype(np.float64)
    px = xx + (kw - pad) + dx.astype(np.float64)
    y0 = np.floor(py)
    x0 = np.floor(px)
    fy = (py - y0).astype(np.float32)
    fx = (px - x0).astype(np.float32)
    y0 = y0.astype(np.int32)
    x0 = x0.astype(np.int32)
    # corners outside the padded canvas are exactly zero in the reference
    # (zero padding): zero their weights and clamp addresses into range.
    in_y0 = (y0 >= -PAD) & (y0 <= H + PAD - 1)
    in_y1 = (y0 + 1 >= -PAD) & (y0 + 1 <= H + PAD - 1)
    in_x0 = (x0 >= -PAD) & (x0 <= W + PAD - 1)
    in_x1 = (x0 + 1 >= -PAD) & (x0 + 1 <= W + PAD - 1)
    y0c = np.clip(y0, -PAD, H + PAD - 2)
    x0c = np.clip(x0, -PAD, W + PAD - 2)
    q00 = (y0c + PAD) * WP + (x0c + PAD)
    w00 = (1 - fy) * (1 - fx) * (in_y0 & in_x0)
    w01 = (1 - fy) * fx * (in_y0 & in_x1)
    w10 = fy * (1 - fx) * (in_y1 & in_x0)
    w11 = fy * fx * (in_y1 & in_x1)
    w4 = np.stack([w00, w01, w10, w11], axis=-1).astype(np.float32)
    return q00, w4


def _precompute_layer(off_l, pad):
    KK2 = off_l.shape[0] // 2
    qs, ws = [], []
    for k in range(KK2):
        q00, w4 = _tap_indices_weights(off_l, k, pad)
        qs.append(q00.reshape(-1))
        ws.append(w4.reshape(-1, 4))
    return np.stack(qs), np.stack(ws)


def _pad_image(a):
    C = a.shape[0]
    ap = np.zeros((C, HP, WP), a.dtype)
    ap[:, PAD:PAD + H, PAD:PAD + W] = a.reshape(C, H, W)
    return ap.reshape(C, NPIX_PAD)


def _host_l0(x_n, off0_n, w0, b0):
    q00, w4 = _tap_indices_weights(off0_n, 0, 0)
    q00 = q00.reshape(-1)
    w4 = w4.reshape(-1, 4)
    xp = _pad_image(x_n.astype(np.float32))
    s = (xp[:, q00] * w4[None, :, 0] + xp[:, q00 + 1] * w4[None, :, 1]
         + xp[:, q00 + WP] * w4[None, :, 2] + xp[:, q00 + WP + 1] * w4[None, :, 3])
    out = w0.reshape(w0.shape[0], -1) @ s + b0[:, None]
    return np.maximum(out, 0.0)


def _wrap_idx(idx):
    """ap_gather layout: index j -> partition 16k + j%16, col j//16, all 8 cores same."""
    n = len(idx)
    w = np.zeros((128, n // 16), dtype=np.int16)
    cols = idx.reshape(n // 16, 16)
    for k in range(8):
        w[16 * k:16 * k + 16, :] = cols.T
    return w


# ---------------- device program ----------------

_CIN = {1: 256, 2: 128, 3: 128, 4: 128, 5: 128, 6: 128, 7: 128}


def _build_program():
    nc = bacc.Bacc("TRN2", target_bir_lowering=False, debug=False, num_devices=NCORES)
    f32 = mybir.dt.float32
    bft = mybir.dt.bfloat16
    i16 = mybir.dt.int16

    # combined pair-split constant blob: [A1 half | const half]
    A1_ELEMS = 2 * 128 * PXH                     # 524288
    WT_E = {l: (_CIN[l] // 128) * NTAPS * 128 * 128 for l in range(1, 8)}
    WQ_E = NTAPS * PXH * 4
    _off, CONST_OFF, CONST_ROW = 0, {}, {}
    for l in range(1, 8):
        if l == 4:
            _h0 = _off
            _off = 0
        CONST_ROW[l] = 0 if l < 4 else 1
        CONST_OFF[l] = _off
        _off += WT_E[l]
    CONST_HALF = max(_h0, _off)                  # padded equal halves
    WT_CHUNK = 147456                            # one l2-7 layer's wt; l1 = 2 chunks
    CB_ROW = A1_ELEMS + CONST_HALF
    a_CB = nc.dram_tensor("CB", (1, A1_ELEMS), bft, kind="ExternalInput").ap()
    cc_in0 = nc.dram_tensor("cc_in0", (1, A1_ELEMS), bft, kind="Internal").ap()
    cc_out0 = nc.dram_tensor("cc_out0", (2, A1_ELEMS), bft, kind="Internal").ap()
    a_WT = nc.dram_tensor("WTC", (1, WT_CHUNK), bft, kind="ExternalInput").ap()
    wt_in = nc.dram_tensor("wt_in", (1, WT_CHUNK), bft, kind="Internal").ap()
    wt_all = nc.dram_tensor("wt_all", (8, WT_CHUNK), bft, kind="Internal").ap()
    a_idx, a_wq, a_wt, a_bias = {}, {}, {}, {}
    cc_in, cc_out = {}, {}
    for l in range(1, 8):
        nblk = _CIN[l] // 128
        a_idx[l] = nc.dram_tensor(f"idx{l}", (16, 3 * (NI_CHUNK // 16)), i16, kind="ExternalInput").ap()
        a_wq[l] = nc.dram_tensor(f"wq{l}", (1, NTAPS * PXH * 2), bft, kind="ExternalInput").ap()
        a_bias[l] = nc.dram_tensor(f"bias{l}", (128, 1), f32, kind="ExternalInput").ap()
        if l < 7:
            cc_in[l] = nc.dram_tensor(f"cc_in{l}", (1, 128 * PXH), bft, kind="Internal").ap()
            cc_out[l] = nc.dram_tensor(f"cc_out{l}", (2, 128 * PXH), bft, kind="Internal").ap()
    a_y = nc.dram_tensor("y", (128, PXH), f32, kind="ExternalOutput").ap()

    with tile.TileContext(nc, num_cores=NCORES) as tc, ExitStack() as ctx:
        apool = ctx.enter_context(tc.tile_pool(name="apad", bufs=2))
        q4pool = ctx.enter_context(tc.tile_pool(name="q4", bufs=1))
        gpool = ctx.enter_context(tc.tile_pool(name="g", bufs=1))
        wqpool = ctx.enter_context(tc.tile_pool(name="wqr", bufs=1))
        wbpool = ctx.enter_context(tc.tile_pool(name="wb", bufs=1))
        bkpool = ctx.enter_context(tc.tile_pool(name="bk", bufs=1))
        wtpool = ctx.enter_context(tc.tile_pool(name="wt", bufs=2))
        idxpool = ctx.enter_context(tc.tile_pool(name="idx", bufs=2))
        evpool = ctx.enter_context(tc.tile_pool(name="ev", bufs=2))
        mpool = ctx.enter_context(tc.tile_pool(name="misc", bufs=1))
        pspool = ctx.enter_context(tc.tile_pool(name="ps", bufs=1, space="PSUM"))

        # reconstruct full A1 (pair) + all conv weights (8-way)
        t_sw = gpool.tile([128, WT_CHUNK // 128], bft, tag="g")
        nc.sync.dma_start(t_sw[:], a_WT[:].rearrange("o (p q) -> (o p) q", p=128))
        nc.sync.dma_start(wt_in[:].rearrange("o (p q) -> (o p) q", p=128), t_sw[:])
        nc.gpsimd.collective_compute(
            "AllGather", mybir.AluOpType.bypass,
            replica_groups=[[0, 1, 2, 3, 4, 5, 6, 7]],
            ins=[wt_in[:]], outs=[wt_all[:]])
        t_st = q4pool.tile([128, A1_ELEMS // 128], bft, tag="q4")
        nc.sync.dma_start(t_st[:], a_CB[:].rearrange("o (p q) -> (o p) q", p=128))
        nc.sync.dma_start(cc_in0[:].rearrange("o (p q) -> (o p) q", p=128), t_st[:])
        nc.gpsimd.collective_compute(
            "AllGather", mybir.AluOpType.bypass,
            replica_groups=[[0, 1], [2, 3], [4, 5], [6, 7]],
            ins=[cc_in0[:]], outs=[cc_out0[:]])
        apad_next = []  # tiles holding next layer's input blocks
        cc0_v = cc_out0[:].rearrange("h (b c y x) -> h b c y x", b=2, c=128, y=H // 2)
        for blk in range(2):
            t = apool.tile([128, NPIX_PAD], bft, tag="apad")
            nc.vector.memset(t[:], 0.0)
            t3 = t[:].rearrange("p (y x) -> p y x", y=HP)
            for h in range(2):
                nc.sync.dma_start(
                    t3[:, PAD + 32 * h:PAD + 32 * h + 32, PAD:PAD + W],
                    cc0_v[h, blk])
            apad_next.append(t)

        for l in range(1, 8):
            nblk = _CIN[l] // 128
            apads = apad_next

            t_idx = idxpool.tile([128, 3 * (NI_CHUNK // 16)], i16, tag="idx")
            for g in range(8):
                nc.sync.dma_start(t_idx[16 * g:16 * g + 16, :], a_idx[l][:])
            t_wt = wtpool.tile([128, nblk * NTAPS * 128], bft, tag="wt")
            if l == 1:
                wt_src = wt_all[0:2, :].rearrange("a (t p m) -> (a t) p m", p=128, m=128)
            else:
                wt_src = wt_all[l, :].rearrange("(t p m) -> t p m", p=128, m=128)
            nc.sync.dma_start(
                t_wt[:].rearrange("p (t m) -> p t m", m=128),
                wt_src.transpose([1, 0, 2]))
            t_bias = mpool.tile([128, 1], f32, tag="bias")
            nc.sync.dma_start(t_bias[:], a_bias[l][:])

            t_ps = pspool.tile([128, PXH], f32, tag="psacc")
            n_mm = nblk * NTAPS * 4
            mm_i = 0
            for blk in range(nblk):
                # Q4 pack: [128, q, dy, dx] <- A_pad[q + {0,1,WP,WP+1}]
                t_q4 = q4pool.tile([128, NPIX_PAD * 4], bft, tag="q4")
                src = apads[blk][:]
                src_view = bass.AP(
                    tensor=src.tensor, offset=src.offset,
                    ap=[list(src.ap[0]), [1, Q4_BUILD], [WP, 2], [1, 2]])
                dst = t_q4[:]
                dst_view = bass.AP(
                    tensor=dst.tensor, offset=dst.offset,
                    ap=[list(dst.ap[0]), [4, Q4_BUILD], [2, 2], [1, 2]])
                nc.vector.tensor_copy(dst_view, src_view)
                for chunk in range(3):
                    t_g = gpool.tile([128, NI_CHUNK * 4], bft, tag="g")
                    nc.gpsimd.ap_gather(
                        t_g[:], t_q4[:],
                        t_idx[:, chunk * (NI_CHUNK // 16):(chunk + 1) * (NI_CHUNK // 16)],
                        channels=128, num_elems=NPIX_PAD, d=4, num_idxs=NI_CHUNK)
                    for t in range(CHUNK_TAPS):
                        k = CHUNK_TAPS * chunk + t
                        t_wq = wqpool.tile([1, PXH * 4], bft, tag="wqr")
                        t_f = mpool.tile([1, PXH * 2], bft, tag="fxy")
                        nc.sync.dma_start(t_f[:], a_wq[l][:, k * PXH * 2:(k + 1) * PXH * 2])
                        fx, fy = t_f[:, :PXH], t_f[:, PXH:]
                        w4v = t_wq[:].rearrange("o (q j) -> o q j", j=4)
                        # build weights using w4 slots as scratch (gx->slot0, gy->slot1)
                        nc.vector.tensor_scalar(w4v[:, :, 0], fx, -1.0, 1.0,
                                                op0=mybir.AluOpType.mult, op1=mybir.AluOpType.add)
                        nc.vector.tensor_scalar(w4v[:, :, 1], fy, -1.0, 1.0,
                                                op0=mybir.AluOpType.mult, op1=mybir.AluOpType.add)
                        nc.vector.tensor_mul(w4v[:, :, 3], fy, fx)
                        nc.vector.tensor_mul(w4v[:, :, 2], fy, w4v[:, :, 0])
                        nc.vector.tensor_mul(w4v[:, :, 0], w4v[:, :, 1], w4v[:, :, 0])
                        nc.vector.tensor_mul(w4v[:, :, 1], w4v[:, :, 1], fx)
                        t_wb = wbpool.tile([128, PXH * 4], bft, tag="wb")
                        nc.gpsimd.partition_broadcast(t_wb[:], t_wq[:])
                        g_slice = t_g[:, t * PXH * 4:(t + 1) * PXH * 4]
                        nc.vector.tensor_mul(g_slice, g_slice, t_wb[:])
                        t_bk = bkpool.tile([128, PXH], bft, tag="bk")
                        with nc.allow_low_precision("bf16 im2col"):
                            nc.vector.tensor_reduce(
                                t_bk[:],
                                g_slice.rearrange("p (q j) -> p q j", j=4),
                                axis=mybir.AxisListType.X, op=mybir.AluOpType.add)
                        lhsT = t_wt[:, (blk * NTAPS + k) * 128:(blk * NTAPS + k + 1) * 128]
                        first = (blk == 0 and k == 0)
                        last = (blk == nblk - 1 and k == NTAPS - 1)
                        for nck in range(4):
                            nc.tensor.matmul(
                                t_ps[:, nck * 512:(nck + 1) * 512],
                                lhsT, t_bk[:, nck * 512:(nck + 1) * 512],
                                start=first, stop=last)
                            mm_i += 1

            # eviction: relu(psum + bias)
            if l < 7:
                t_ev = evpool.tile([128, PXH], bft, tag="ev")
            else:
                t_ev = evpool.tile([128, PXH], f32, tag="ev7")
            nc.scalar.activation(t_ev[:], t_ps[:], mybir.ActivationFunctionType.Relu,
                                 bias=t_bias[:], scale=1.0)

            if l < 7:
                nc.sync.dma_start(
                    cc_in[l][:].rearrange("o (p q) -> (o p) q", p=128), t_ev[:])
                nc.gpsimd.collective_compute(
                    "AllGather", mybir.AluOpType.bypass,
                    replica_groups=[[0, 1], [2, 3], [4, 5], [6, 7]],
                    ins=[cc_in[l][:]], outs=[cc_out[l][:]])
                t_an = apool.tile([128, NPIX_PAD], bft, tag="apad")
                nc.vector.memset(t_an[:], 0.0)
                an3 = t_an[:].rearrange("p (y x) -> p y x", y=HP)
                cc3 = cc_out[l][:].rearrange("h (c y x) -> h c y x", c=128, y=H // 2)
                for h in range(2):
                    nc.sync.dma_start(
                        an3[:, PAD + 32 * h:PAD + 32 * h + 32, PAD:PAD + W],
                        cc3[h])
                apad_next = [t_an]
            else:
                nc.sync.dma_start(a_y[:], t_ev[:])

    nc.compile()
    return nc


# ---------------- entry point ----------------

_LAST_RUN_NS = None


def kernel(**inputs):
    global _LAST_RUN_NS
    _t0 = _time.time()
    inputs = {k: np.asarray(v) for k, v in inputs.items()}
    x = inputs["x"].astype(np.float32)
    N = x.shape[0]
    assert N * 2 == NCORES

    # layer 0 on host
    A1 = np.stack([
        _host_l0(x[n], np.asarray(inputs["off0"][n], np.float32),
                 np.asarray(inputs["w0"], np.float32),
                 np.asarray(inputs["b0"], np.float32))
        for n in range(N)])                      # [N, 256, NPIX] f32

    _t1 = _time.time()
    nc = _build_program()
    _t2 = _time.time()

    in_maps = []
    for core in range(NCORES):
        s, h = core // 2, core % 2
        m = {}
        px_sel = slice(h * PXH, (h + 1) * PXH)   # row-major half
        const_parts = []
        for l in range(1, 8):
            q00, w4 = _precompute_layer(np.asarray(inputs[f"off{l}"][s], np.float32), 1)
            qh = q00[:, px_sel]                  # [9, 2048]
            wh = w4[:, px_sel, :]                # [9, 2048, 4]
            assert qh.max() < Q4_BUILD
            idx_chunks = [
                qh[c * CHUNK_TAPS:(c + 1) * CHUNK_TAPS].reshape(-1, 16).T.astype(np.int16)
                for c in range(3)]
            m[f"idx{l}"] = np.concatenate(idx_chunks, axis=1)
            assert np.abs(wh.sum(-1) - 1.0).max() < 1e-5, "corner mask active; fx/fy form invalid"
            fxh = wh[:, :, 1] + wh[:, :, 3]      # [9, 2048]
            fyh = wh[:, :, 2] + wh[:, :, 3]
            m[f"wq{l}"] = np.stack([fxh, fyh], axis=1).reshape(1, -1).astype(bf16)
            wl = np.asarray(inputs[f"w{l}"], np.float32)   # [128, cin, 3, 3]
            nblk = _CIN[l] // 128
            wt = np.empty((nblk * NTAPS, 128, 128), bf16)
            for blk in range(nblk):
                for k in range(NTAPS):
                    kh, kw = divmod(k, K)
                    wt[blk * NTAPS + k] = wl[:, blk * 128:(blk + 1) * 128, kh, kw].T.astype(bf16)
            const_parts.append(wt.reshape(-1))
            m[f"bias{l}"] = np.asarray(inputs[f"b{l}"], np.float32).reshape(128, 1)
        m["CB"] = A1[s][:, px_sel].astype(bf16).reshape(1, -1)
        wt_flat = np.concatenate(const_parts)    # all 8 chunks, built below
        m["WTC"] = wt_flat[core * 147456:(core + 1) * 147456].reshape(1, -1)
        in_maps.append(m)

    _t3 = _time.time()
    res = bass_utils.run_bass_kernel_spmd(nc, in_maps, core_ids=list(range(NCORES)))
    _t4 = _time.time()
    _LAST_RUN_NS = int((_t4 - _t3) * 1e9)
    print(f"[kernel] host_l0={_t1-_t0:.2f}s build={_t2-_t1:.2f}s prep={_t3-_t2:.2f}s "
          f"run={_t4-_t3:.2f}s")

    out = np.empty((N, 128, H, W), np.float32)
    for core in range(NCORES):
        s, h = core // 2, core % 2
        y = res.results[core]["y"]               # [128, 2048]
        out[s, :, 32 * h:32 * h + 32, :] = y.reshape(128, 32, W)
    return out



# revision 6
# speedup vs baseline: 1.7517x; 1.7517x over previous
"""Deformable-conv stack (8 layers) on 8 Trainium2 NeuronCores.

Strategy:
  - Layer 0 (1x1 deform conv, 512->256) computed on host (x and off0 are
    kernel inputs, so the sampled im2col and the 1x1 conv are host numpy).
  - Layers 1..7 (3x3 deform convs) on device, data-parallel over
    (sample, image-half): core 2s+h handles rows 32h..32h+31 of sample s.
  - All sampling indices / bilinear weights precomputed on host.
  - Host<->device traffic is the bottleneck (axon tunnel ~40MB/s), so
    inputs are consolidated into 4 tensors and quantized:
      * A1 activations: u8 with per-channel scale (scale applied on device)
      * bilinear fractions fx/fy: u8 (/255)
      * conv weights: bf16, sharded 8-way + device AllGather
      * output: f16
  - Device per layer: pack Q4 (4 corners interleaved, padded 80x80 image),
    ap_gather per 3-tap chunk, DVE multiply by broadcast bilinear weights +
    inner-4 reduce -> im2col slice, PE matmuls accumulate in PSUM,
    ACT relu+bias eviction, pair AllGather to rebuild the full image.
"""
import time as _time
import numpy as np
import ml_dtypes
from contextlib import ExitStack

import concourse.bass as bass
import concourse.mybir as mybir
import concourse.tile as tile
from concourse import bass_utils
from concourse import bacc

bf16 = ml_dtypes.bfloat16

H = W = 64
PAD = 8
HP = WP = H + 2 * PAD          # 80
NPIX_PAD = HP * WP             # 6400
Q4_BUILD = (HP - 2) * WP + (WP - 2) + 1   # max valid q00 + 1
NPIX = H * W
PXH = NPIX // 2                # 2048
K = 3
NCORES = 8
NTAPS = 9
CHUNK_TAPS = 3
NI_CHUNK = CHUNK_TAPS * PXH    # 6144 indices per gather

A1_ELEMS = 2 * 128 * PXH       # 524288 (u8 bytes)
FXY_PER_TAP = PXH * 2          # 4096 (fx | fy)
FXY_ELEMS = 7 * NTAPS * FXY_PER_TAP   # 258048
U8_TOTAL = A1_ELEMS + FXY_ELEMS       # 782336
IDX_COLS = 3 * (NI_CHUNK // 16)       # 1152 per layer
WT_CHUNK = 147456              # per-core weight shard (bf16 elems)
F32_TOTAL = 256 + 7 * 128      # a1 scales | biases


# ---------------- host-side index/weight precompute ----------------

def _tap_indices_weights(off_l, k, pad):
    KK = int(round(np.sqrt(off_l.shape[0] // 2)))
    kh, kw = divmod(k, KK)
    dy = off_l[2 * k]
    dx = off_l[2 * k + 1]
    yy = np.arange(H, dtype=np.float64)[:, None]
    xx = np.arange(W, dtype=np.float64)[None, :]
    py = yy + (kh - pad) + dy.astype(np.float64)
    px = xx + (kw - pad) + dx.astype(np.float64)
    y0 = np.floor(py)
    x0 = np.floor(px)
    fy = (py - y0).astype(np.float32)
    fx = (px - x0).astype(np.float32)
    y0 = y0.astype(np.int32)
    x0 = x0.astype(np.int32)
    # corners outside the padded canvas are exactly zero in the reference
    # (zero padding): zero their weights and clamp addresses into range.
    in_y0 = (y0 >= -PAD) & (y0 <= H + PAD - 1)
    in_y1 = (y0 + 1 >= -PAD) & (y0 + 1 <= H + PAD - 1)
    in_x0 = (x0 >= -PAD) & (x0 <= W + PAD - 1)
    in_x1 = (x0 + 1 >= -PAD) & (x0 + 1 <= W + PAD - 1)
    y0c = np.clip(y0, -PAD, H + PAD - 2)
    x0c = np.clip(x0, -PAD, W + PAD - 2)
    q00 = (y0c + PAD) * WP + (x0c + PAD)
    w00 = (1 - fy) * (1 - fx) * (in_y0 & in_x0)
    w01 = (1 - fy) * fx * (in_y0 & in_x1)
    w10 = fy * (1 - fx) * (in_y1 & in_x0)
    w11 = fy * fx * (in_y1 & in_x1)
    w4 = np.stack([w00, w01, w10, w11], axis=-1).astype(np.float32)
    return q00, w4


def _precompute_layer(off_l, pad):
    KK2 = off_l.shape[0] // 2
    qs, ws = [], []
    for k in range(KK2):
        q00, w4 = _tap_indices_weights(off_l, k, pad)
        qs.append(q00.reshape(-1))
        ws.append(w4.reshape(-1, 4))
    return np.stack(qs), np.stack(ws)


def _pad_image(a):
    C = a.shape[0]
    ap = np.zeros((C, HP, WP), a.dtype)
    ap[:, PAD:PAD + H, PAD:PAD + W] = a.reshape(C, H, W)
    return ap.reshape(C, NPIX_PAD)


def _host_l0(x_n, off0_n, w0, b0):
    q00, w4 = _tap_indices_weights(off0_n, 0, 0)
    q00 = q00.reshape(-1)
    w4 = w4.reshape(-1, 4)
    xp = _pad_image(x_n.astype(np.float32))
    s = (xp[:, q00] * w4[None, :, 0] + xp[:, q00 + 1] * w4[None, :, 1]
         + xp[:, q00 + WP] * w4[None, :, 2] + xp[:, q00 + WP + 1] * w4[None, :, 3])
    out = w0.reshape(w0.shape[0], -1) @ s + b0[:, None]
    return np.maximum(out, 0.0)


# ---------------- device program ----------------

_CIN = {1: 256, 2: 128, 3: 128, 4: 128, 5: 128, 6: 128, 7: 128}


def _build_program():
    nc = bacc.Bacc("TRN2", target_bir_lowering=False, debug=False, num_devices=NCORES)
    f32 = mybir.dt.float32
    bft = mybir.dt.bfloat16
    i16 = mybir.dt.int16
    u8 = mybir.dt.uint8
    f16 = mybir.dt.float16

    a_U8 = nc.dram_tensor("U8", (1, U8_TOTAL), u8, kind="ExternalInput").ap()
    a_I16 = nc.dram_tensor("IX", (16, 7 * IDX_COLS), i16, kind="ExternalInput").ap()
    a_WT = nc.dram_tensor("WTC", (1, WT_CHUNK), bft, kind="ExternalInput").ap()
    a_F32 = nc.dram_tensor("FB", (1, F32_TOTAL), f32, kind="ExternalInput").ap()

    cc_in0 = nc.dram_tensor("cc_in0", (1, A1_ELEMS), u8, kind="Internal").ap()
    cc_out0 = nc.dram_tensor("cc_out0", (2, A1_ELEMS), u8, kind="Internal").ap()
    wt_in = nc.dram_tensor("wt_in", (1, WT_CHUNK), bft, kind="Internal").ap()
    wt_all = nc.dram_tensor("wt_all", (8, WT_CHUNK), bft, kind="Internal").ap()
    cc_in, cc_out = {}, {}
    for l in range(1, 7):
        cc_in[l] = nc.dram_tensor(f"cc_in{l}", (1, 128 * PXH), bft, kind="Internal").ap()
        cc_out[l] = nc.dram_tensor(f"cc_out{l}", (2, 128 * PXH), bft, kind="Internal").ap()
    a_y = nc.dram_tensor("y", (128, PXH), f16, kind="ExternalOutput").ap()

    with tile.TileContext(nc, num_cores=NCORES) as tc, ExitStack() as ctx:
        apool = ctx.enter_context(tc.tile_pool(name="apad", bufs=2))
        q4pool = ctx.enter_context(tc.tile_pool(name="q4", bufs=1))
        gpool = ctx.enter_context(tc.tile_pool(name="g", bufs=1))
        wqpool = ctx.enter_context(tc.tile_pool(name="wqr", bufs=1))
        wbpool = ctx.enter_context(tc.tile_pool(name="wb", bufs=1))
        bkpool = ctx.enter_context(tc.tile_pool(name="bk", bufs=1))
        wtpool = ctx.enter_context(tc.tile_pool(name="wt", bufs=2))
        idxpool = ctx.enter_context(tc.tile_pool(name="idx", bufs=2))
        evpool = ctx.enter_context(tc.tile_pool(name="ev", bufs=2))
        mpool = ctx.enter_context(tc.tile_pool(name="misc", bufs=1))
        pspool = ctx.enter_context(tc.tile_pool(name="ps", bufs=1, space="PSUM"))

        # weights: shard -> AllGather (8-way)
        t_sw = gpool.tile([128, WT_CHUNK // 128], bft, tag="g")
        nc.sync.dma_start(t_sw[:], a_WT[:].rearrange("o (p q) -> (o p) q", p=128))
        nc.sync.dma_start(wt_in[:].rearrange("o (p q) -> (o p) q", p=128), t_sw[:])
        nc.gpsimd.collective_compute(
            "AllGather", mybir.AluOpType.bypass,
            replica_groups=[[0, 1, 2, 3, 4, 5, 6, 7]],
            ins=[wt_in[:]], outs=[wt_all[:]])
        # A1 u8 half: -> AllGather (pairs)
        t_st = q4pool.tile([128, A1_ELEMS // 128], u8, tag="q4")
        nc.sync.dma_start(t_st[:], a_U8[:, :A1_ELEMS].rearrange("o (p q) -> (o p) q", p=128))
        nc.sync.dma_start(cc_in0[:].rearrange("o (p q) -> (o p) q", p=128), t_st[:])
        nc.gpsimd.collective_compute(
            "AllGather", mybir.AluOpType.bypass,
            replica_groups=[[0, 1], [2, 3], [4, 5], [6, 7]],
            ins=[cc_in0[:]], outs=[cc_out0[:]])
        # per-channel A1 scales
        t_scl = mpool.tile([128, 2], f32, tag="scl")
        nc.sync.dma_start(t_scl[:], a_F32[:, :256].rearrange("o (b p) -> (o p) b", p=128))

        # build padded canvases: u8 staging -> ACT copy*scale -> bf16 interior
        apad_next = []
        cc0_f = cc_out0[:].rearrange("h (b c q) -> h b c q", b=2, c=128)
        for blk in range(2):
            t = apool.tile([128, NPIX_PAD], bft, tag="apad")
            nc.vector.memset(t[:], 0.0)
            t_s = q4pool.tile([128, NPIX], u8, tag="q4")
            for h in range(2):
                nc.sync.dma_start(t_s[:, h * PXH:(h + 1) * PXH], cc0_f[h, blk])
            t3 = t[:].rearrange("p (y x) -> p y x", y=HP)
            nc.scalar.activation(
                t3[:, PAD:PAD + H, PAD:PAD + W],
                t_s[:].rearrange("p (y x) -> p y x", y=H),
                mybir.ActivationFunctionType.Copy,
                scale=t_scl[:, blk:blk + 1])
            apad_next.append(t)

        for l in range(1, 8):
            nblk = _CIN[l] // 128
            apads = apad_next

            t_idx = idxpool.tile([128, IDX_COLS], i16, tag="idx")
            for g in range(8):
                nc.sync.dma_start(t_idx[16 * g:16 * g + 16, :],
                                  a_I16[:, (l - 1) * IDX_COLS:l * IDX_COLS])
            t_wt = wtpool.tile([128, nblk * NTAPS * 128], bft, tag="wt")
            if l == 1:
                wt_src = wt_all[0:2, :].rearrange("a (t p m) -> (a t) p m", p=128, m=128)
            else:
                wt_src = wt_all[l, :].rearrange("(t p m) -> t p m", p=128, m=128)
            nc.sync.dma_start(
                t_wt[:].rearrange("p (t m) -> p t m", m=128),
                wt_src.transpose([1, 0, 2]))
            t_bias = mpool.tile([128, 1], f32, tag="bias")
            nc.sync.dma_start(
                t_bias[:],
                a_F32[:, 256 + (l - 1) * 128:256 + l * 128].rearrange(
                    "o (p q) -> (o p) q", p=128))

            t_ps = pspool.tile([128, PXH], f32, tag="psacc")
            for blk in range(nblk):
                # Q4 pack: [128, q, dy, dx] <- A_pad[q + {0,1,WP,WP+1}]
                t_q4 = q4pool.tile([128, NPIX_PAD * 4], bft, tag="q4")
                src = apads[blk][:]
                src_view = bass.AP(
                    tensor=src.tensor, offset=src.offset,
                    ap=[list(src.ap[0]), [1, Q4_BUILD], [WP, 2], [1, 2]])
                dst = t_q4[:]
                dst_view = bass.AP(
                    tensor=dst.tensor, offset=dst.offset,
                    ap=[list(dst.ap[0]), [4, Q4_BUILD], [2, 2], [1, 2]])
                nc.vector.tensor_copy(dst_view, src_view)
                for chunk in range(3):
                    t_g = gpool.tile([128, NI_CHUNK * 4], bft, tag="g")
                    nc.gpsimd.ap_gather(
                        t_g[:], t_q4[:, :4 * Q4_BUILD],
                        t_idx[:, chunk * (NI_CHUNK // 16):(chunk + 1) * (NI_CHUNK // 16)],
                        channels=128, num_elems=Q4_BUILD, d=4, num_idxs=NI_CHUNK)
                    for t in range(CHUNK_TAPS):
                        k = CHUNK_TAPS * chunk + t
                        t_wq = wqpool.tile([1, PXH * 4], bft, tag="wqr")
                        t_fu8 = mpool.tile([1, FXY_PER_TAP], u8, tag="fu8")
                        nc.sync.dma_start(
                            t_fu8[:],
                            a_U8[:, A1_ELEMS + ((l - 1) * NTAPS + k) * FXY_PER_TAP:
                                 A1_ELEMS + ((l - 1) * NTAPS + k + 1) * FXY_PER_TAP])
                        fxu8, fyu8 = t_fu8[:, :PXH], t_fu8[:, PXH:]
                        w4v = t_wq[:].rearrange("o (q j) -> o q j", j=4)
                        # weight build in-place ([0]=w00 [1]=w01 [2]=w10 [3]=w11):
                        #   s1=fx, s2=fy, s3=fx*fy, s0=1-fx,
                        #   s2=fy*(1-fx), s1=fx-fx*fy, s0=(1-fx)-fy*(1-fx)
                        nc.vector.tensor_scalar(w4v[:, :, 1], fxu8, 1.0 / 255.0, None,
                                                op0=mybir.AluOpType.mult)
                        nc.vector.tensor_scalar(w4v[:, :, 2], fyu8, 1.0 / 255.0, None,
                                                op0=mybir.AluOpType.mult)
                        nc.vector.tensor_mul(w4v[:, :, 3], w4v[:, :, 2], w4v[:, :, 1])
                        nc.vector.tensor_scalar(w4v[:, :, 0], fxu8, -1.0 / 255.0, 1.0,
                                                op0=mybir.AluOpType.mult, op1=mybir.AluOpType.add)
                        nc.vector.tensor_mul(w4v[:, :, 2], w4v[:, :, 2], w4v[:, :, 0])
                        nc.vector.tensor_sub(w4v[:, :, 1], w4v[:, :, 1], w4v[:, :, 3])
                        nc.vector.tensor_sub(w4v[:, :, 0], w4v[:, :, 0], w4v[:, :, 2])
                        t_wb = wbpool.tile([128, PXH * 4], bft, tag="wb")
                        nc.gpsimd.partition_broadcast(t_wb[:], t_wq[:])
                        g_slice = t_g[:, t * PXH * 4:(t + 1) * PXH * 4]
                        nc.vector.tensor_mul(g_slice, g_slice, t_wb[:])
                        t_bk = bkpool.tile([128, PXH], bft, tag="bk")
                        with nc.allow_low_precision("bf16 im2col"):
                            nc.vector.tensor_reduce(
                                t_bk[:],
                                g_slice.rearrange("p (q j) -> p q j", j=4),
                                axis=mybir.AxisListType.X, op=mybir.AluOpType.add)
                        lhsT = t_wt[:, (blk * NTAPS + k) * 128:(blk * NTAPS + k + 1) * 128]
                        first = (blk == 0 and k == 0)
                        last = (blk == nblk - 1 and k == NTAPS - 1)
                        for nck in range(4):
                            nc.tensor.matmul(
                                t_ps[:, nck * 512:(nck + 1) * 512],
                                lhsT, t_bk[:, nck * 512:(nck + 1) * 512],
                                start=first, stop=last)

            # eviction: relu(psum + bias)
            if l < 7:
                t_ev = evpool.tile([128, PXH], bft, tag="ev")
            else:
                t_ev = evpool.tile([128, PXH], f16, tag="ev7")
            nc.scalar.activation(t_ev[:], t_ps[:], mybir.ActivationFunctionType.Relu,
                                 bias=t_bias[:], scale=1.0)

            if l < 7:
                nc.sync.dma_start(
                    cc_in[l][:].rearrange("o (p q) -> (o p) q", p=128), t_ev[:])
                nc.gpsimd.collective_compute(
                    "AllGather", mybir.AluOpType.bypass,
                    replica_groups=[[0, 1], [2, 3], [4, 5], [6, 7]],
                    ins=[cc_in[l][:]], outs=[cc_out[l][:]])
                t_an = apool.tile([128, NPIX_PAD], bft, tag="apad")
                nc.vector.memset(t_an[:], 0.0)
                an3 = t_an[:].rearrange("p (y x) -> p y x", y=HP)
                cc3 = cc_out[l][:].rearrange("h (c y x) -> h c y x", c=128, y=H // 2)
                for h in range(2):
                    nc.sync.dma_start(
                        an3[:, PAD + 32 * h:PAD + 32 * h + 32, PAD:PAD + W],
                        cc3[h])
                apad_next = [t_an]
            else:
                nc.sync.dma_start(a_y[:], t_ev[:])

    nc.compile()
    return nc


# ---------------- host input prep ----------------

def _prep_in_maps(inputs, A1):
    """Build per-core consolidated input tensors."""
    in_maps = []
    N = A1.shape[0]
    # per-sample A1 u8 quantization (scales shared by the core pair)
    a1_q, a1_scale = [], []
    for s in range(N):
        mx = np.maximum(A1[s].max(axis=1), 1e-12).astype(np.float32)
        q = np.rint(A1[s] * (255.0 / mx)[:, None]).astype(np.uint8)
        a1_q.append(q)
        a1_scale.append((mx / 255.0).astype(np.float32))
    # per-core weight shard (same for all cores; shard by core id)
    const_parts = []
    for l in range(1, 8):
        wl = np.asarray(inputs[f"w{l}"], np.float32)
        nblk = _CIN[l] // 128
        wt = np.empty((nblk * NTAPS, 128, 128), bf16)
        for blk in range(nblk):
            for k in range(NTAPS):
                kh, kw = divmod(k, K)
                wt[blk * NTAPS + k] = wl[:, blk * 128:(blk + 1) * 128, kh, kw].T.astype(bf16)
        const_parts.append(wt.reshape(-1))
    wt_flat = np.concatenate(const_parts)
    biases = np.concatenate([np.asarray(inputs[f"b{l}"], np.float32) for l in range(1, 8)])

    for core in range(NCORES):
        s, h = core // 2, core % 2
        px_sel = slice(h * PXH, (h + 1) * PXH)
        u8_parts = [a1_q[s][:, px_sel].reshape(-1)]
        idx_parts = []
        for l in range(1, 8):
            q00, w4 = _precompute_layer(np.asarray(inputs[f"off{l}"][s], np.float32), 1)
            qh = q00[:, px_sel]                  # [9, 2048]
            wh = w4[:, px_sel, :]                # [9, 2048, 4]
            assert qh.max() < Q4_BUILD
            idx_chunks = [
                qh[c * CHUNK_TAPS:(c + 1) * CHUNK_TAPS].reshape(-1, 16).T.astype(np.int16)
                for c in range(3)]
            idx_parts.append(np.concatenate(idx_chunks, axis=1))
            assert np.abs(wh.sum(-1) - 1.0).max() < 1e-5, "corner mask active; fx/fy form invalid"
            fxh = wh[:, :, 1] + wh[:, :, 3]      # [9, 2048]
            fyh = wh[:, :, 2] + wh[:, :, 3]
            fxy = np.stack([fxh, fyh], axis=1)   # [9, 2, 2048]
            u8_parts.append(np.rint(fxy * 255.0).astype(np.uint8).reshape(-1))
        m = {
            "U8": np.concatenate(u8_parts).reshape(1, -1),
            "IX": np.concatenate(idx_parts, axis=1),
            "WTC": wt_flat[core * WT_CHUNK:(core + 1) * WT_CHUNK].reshape(1, -1),
            "FB": np.concatenate([a1_scale[s], biases]).reshape(1, -1),
        }
        in_maps.append(m)
    return in_maps


# ---------------- entry point ----------------

_LAST_RUN_NS = None
_NC_CACHE = None


def kernel(**inputs):
    global _LAST_RUN_NS, _NC_CACHE
    _t0 = _time.time()
    inputs = {k: np.asarray(v) for k, v in inputs.items()}
    x = inputs["x"].astype(np.float32)
    N = x.shape[0]
    assert N * 2 == NCORES

    # layer 0 on host
    A1 = np.stack([
        _host_l0(x[n], np.asarray(inputs["off0"][n], np.float32),
                 np.asarray(inputs["w0"], np.float32),
                 np.asarray(inputs["b0"], np.float32))
        for n in range(N)])                      # [N, 256, NPIX] f32

    _t1 = _time.time()
    if _NC_CACHE is None:
        _NC_CACHE = _build_program()
    nc = _NC_CACHE
    _t2 = _time.time()

    in_maps = _prep_in_maps(inputs, A1)

    _t3 = _time.time()
    res = bass_utils.run_bass_kernel_spmd(nc, in_maps, core_ids=list(range(NCORES)))
    _t4 = _time.time()
    _LAST_RUN_NS = int((_t4 - _t3) * 1e9)
    print(f"[kernel] host_l0={_t1-_t0:.2f}s build={_t2-_t1:.2f}s prep={_t3-_t2:.2f}s "
          f"run={_t4-_t3:.2f}s")

    out = np.empty((N, 128, H, W), np.float32)
    for core in range(NCORES):
        s, h = core // 2, core % 2
        y = res.results[core]["y"]               # [128, 2048] f16
        out[s, :, 32 * h:32 * h + 32, :] = y.astype(np.float32).reshape(128, 32, W)
    return out


# revision 12
# speedup vs baseline: 1.8478x; 1.0548x over previous
"""Deformable-conv stack (8 layers) on 8 Trainium2 NeuronCores.

Strategy:
  - Layer 0 (1x1 deform conv, 512->256) computed on host (x and off0 are
    kernel inputs, so the sampled im2col and the 1x1 conv are host numpy).
  - Layers 1..7 (3x3 deform convs) on device, data-parallel over
    (sample, image-half): core 2s+h handles rows 32h..32h+31 of sample s.
  - All sampling indices / bilinear weights precomputed on host.
  - Host<->device traffic is the bottleneck (axon tunnel ~40MB/s), so
    everything rides in ONE u8 input tensor per core (bitcast views):
      * A1 activations: u8 with per-channel scale (scale applied on device)
      * bilinear fractions fx/fy: u8 (/255)
      * conv weights: f16, sharded 8-way + device AllGather
      * gather indices: i16
      * output: u8 with per-channel scale, scales packed into the tensor
  - Device per layer: pack Q4 (4 corners interleaved, padded 80x80 image),
    ap_gather per 3-tap chunk, DVE multiply by broadcast bilinear weights +
    inner-4 reduce -> im2col slice, PE matmuls (f16) accumulate in PSUM,
    ACT relu+bias eviction, pair AllGather to rebuild the full image.
"""
import time as _time
import numpy as np
import ml_dtypes
from contextlib import ExitStack

import concourse.bass as bass
import concourse.mybir as mybir
import concourse.tile as tile
from concourse import bass_utils
from concourse import bacc

bf16 = ml_dtypes.bfloat16
COMPUTE_DT = "f16"   # "f16" | "bf16"

H = W = 64
PAD = 8
HP = WP = H + 2 * PAD          # 80
NPIX_PAD = HP * WP             # 6400
Q4_BUILD = (HP - 2) * WP + (WP - 2) + 1   # max valid q00 + 1
NPIX = H * W
PXH = NPIX // 2                # 2048
K = 3
NCORES = 8
NTAPS = 9
CHUNK_TAPS = 3
NI_CHUNK = CHUNK_TAPS * PXH    # 6144 indices per gather

A1_ELEMS = 2 * 128 * PXH       # 524288 (u8 bytes)
FXY_PER_TAP = PXH * 2          # 4096 (fx | fy)
FXY_ELEMS = 7 * NTAPS * FXY_PER_TAP   # 258048
IDX_COLS = 3 * (NI_CHUNK // 16)       # 1152 per layer
WT_CHUNK = 147456              # per-core weight shard (f16 elems)
F32_ELEMS = 256 + 7 * 128      # a1 scales | biases  (1152)
# single input blob layout (bytes; all sections 4-byte aligned)
OFF_F32 = 0
OFF_WT = OFF_F32 + F32_ELEMS * 4          # 4608
OFF_IDX = OFF_WT + WT_CHUNK * 2           # 299520
OFF_A1 = OFF_IDX + 7 * 16 * IDX_COLS * 2  # 557568
OFF_FXY = OFF_A1 + A1_ELEMS               # 1081856
TOT_BYTES = OFF_FXY + FXY_ELEMS           # 1339904
OUT_COLS = PXH + 4             # 2052: u8 y | f32 channel max (bitcast)


# ---------------- host-side index/weight precompute ----------------

def _tap_indices_weights(off_l, k, pad):
    KK = int(round(np.sqrt(off_l.shape[0] // 2)))
    kh, kw = divmod(k, KK)
    dy = off_l[2 * k]
    dx = off_l[2 * k + 1]
    yy = np.arange(H, dtype=np.float64)[:, None]
    xx = np.arange(W, dtype=np.float64)[None, :]
    py = yy + (kh - pad) + dy.astype(np.float64)
    px = xx + (kw - pad) + dx.astype(np.float64)
    y0 = np.floor(py)
    x0 = np.floor(px)
    fy = (py - y0).astype(np.float32)
    fx = (px - x0).astype(np.float32)
    y0 = y0.astype(np.int32)
    x0 = x0.astype(np.int32)
    # corners outside the padded canvas are exactly zero in the reference
    # (zero padding): zero their weights and clamp addresses into range.
    in_y0 = (y0 >= -PAD) & (y0 <= H + PAD - 1)
    in_y1 = (y0 + 1 >= -PAD) & (y0 + 1 <= H + PAD - 1)
    in_x0 = (x0 >= -PAD) & (x0 <= W + PAD - 1)
    in_x1 = (x0 + 1 >= -PAD) & (x0 + 1 <= W + PAD - 1)
    y0c = np.clip(y0, -PAD, H + PAD - 2)
    x0c = np.clip(x0, -PAD, W + PAD - 2)
    q00 = (y0c + PAD) * WP + (x0c + PAD)
    w00 = (1 - fy) * (1 - fx) * (in_y0 & in_x0)
    w01 = (1 - fy) * fx * (in_y0 & in_x1)
    w10 = fy * (1 - fx) * (in_y1 & in_x0)
    w11 = fy * fx * (in_y1 & in_x1)
    w4 = np.stack([w00, w01, w10, w11], axis=-1).astype(np.float32)
    return q00, w4


def _precompute_layer(off_l, pad):
    KK2 = off_l.shape[0] // 2
    qs, ws = [], []
    for k in range(KK2):
        q00, w4 = _tap_indices_weights(off_l, k, pad)
        qs.append(q00.reshape(-1))
        ws.append(w4.reshape(-1, 4))
    return np.stack(qs), np.stack(ws)


def _pad_image(a):
    C = a.shape[0]
    ap = np.zeros((C, HP, WP), a.dtype)
    ap[:, PAD:PAD + H, PAD:PAD + W] = a.reshape(C, H, W)
    return ap.reshape(C, NPIX_PAD)


def _host_l0(x_n, off0_n, w0, b0):
    q00, w4 = _tap_indices_weights(off0_n, 0, 0)
    q00 = q00.reshape(-1)
    w4 = w4.reshape(-1, 4)
    xp = _pad_image(x_n.astype(np.float32))
    s = (xp[:, q00] * w4[None, :, 0] + xp[:, q00 + 1] * w4[None, :, 1]
         + xp[:, q00 + WP] * w4[None, :, 2] + xp[:, q00 + WP + 1] * w4[None, :, 3])
    out = w0.reshape(w0.shape[0], -1) @ s + b0[:, None]
    return np.maximum(out, 0.0)


# ---------------- device program ----------------

_CIN = {1: 256, 2: 128, 3: 128, 4: 128, 5: 128, 6: 128, 7: 128}


def _build_program():
    nc = bacc.Bacc("TRN2", target_bir_lowering=False, debug=False, num_devices=NCORES)
    f32 = mybir.dt.float32
    i16 = mybir.dt.int16
    u8 = mybir.dt.uint8
    f16 = mybir.dt.float16 if COMPUTE_DT == "f16" else mybir.dt.bfloat16

    h_ALL = nc.dram_tensor("A", (1, TOT_BYTES), u8, kind="ExternalInput")
    a_F32 = h_ALL.bitcast(f32).ap()      # (1, TOT/4)
    a_F16 = h_ALL.bitcast(f16).ap()      # (1, TOT/2)
    a_I16 = h_ALL.bitcast(i16).ap()      # (1, TOT/2)
    a_U8 = h_ALL.ap()

    cc_in0 = nc.dram_tensor("cc_in0", (1, A1_ELEMS), u8, kind="Internal").ap()
    cc_out0 = nc.dram_tensor("cc_out0", (2, A1_ELEMS), u8, kind="Internal").ap()
    wt_in = nc.dram_tensor("wt_in", (1, WT_CHUNK), f16, kind="Internal").ap()
    wt_all = nc.dram_tensor("wt_all", (8, WT_CHUNK), f16, kind="Internal").ap()
    cc_in, cc_out = {}, {}
    for l in range(1, 7):
        cc_in[l] = nc.dram_tensor(f"cc_in{l}", (1, 128 * PXH), f16, kind="Internal").ap()
        cc_out[l] = nc.dram_tensor(f"cc_out{l}", (2, 128 * PXH), f16, kind="Internal").ap()
    a_y = nc.dram_tensor("y", (128, OUT_COLS), u8, kind="ExternalOutput").ap()

    with tile.TileContext(nc, num_cores=NCORES) as tc, ExitStack() as ctx:
        apool = ctx.enter_context(tc.tile_pool(name="apad", bufs=2))
        q4pool = ctx.enter_context(tc.tile_pool(name="q4", bufs=1))
        gpool = ctx.enter_context(tc.tile_pool(name="g", bufs=1))
        wqpool = ctx.enter_context(tc.tile_pool(name="wqr", bufs=1))
        wbpool = ctx.enter_context(tc.tile_pool(name="wb", bufs=1))
        bkpool = ctx.enter_context(tc.tile_pool(name="bk", bufs=1))
        wtpool = ctx.enter_context(tc.tile_pool(name="wt", bufs=2))
        idxpool = ctx.enter_context(tc.tile_pool(name="idx", bufs=2))
        evpool = ctx.enter_context(tc.tile_pool(name="ev", bufs=2))
        mpool = ctx.enter_context(tc.tile_pool(name="misc", bufs=1))
        pspool = ctx.enter_context(tc.tile_pool(name="ps", bufs=1, space="PSUM"))

        # weights: shard -> AllGather (8-way)
        t_sw = gpool.tile([128, WT_CHUNK // 128], f16, tag="g")
        nc.sync.dma_start(
            t_sw[:],
            a_F16[:, OFF_WT // 2:OFF_WT // 2 + WT_CHUNK].rearrange(
                "o (p q) -> (o p) q", p=128))
        nc.sync.dma_start(wt_in[:].rearrange("o (p q) -> (o p) q", p=128), t_sw[:])
        nc.gpsimd.collective_compute(
            "AllGather", mybir.AluOpType.bypass,
            replica_groups=[[0, 1, 2, 3, 4, 5, 6, 7]],
            ins=[wt_in[:]], outs=[wt_all[:]])
        # A1 u8 half -> AllGather (pairs)
        t_st = q4pool.tile([128, A1_ELEMS // 128], u8, tag="q4")
        nc.sync.dma_start(
            t_st[:],
            a_U8[:, OFF_A1:OFF_A1 + A1_ELEMS].rearrange("o (p q) -> (o p) q", p=128))
        nc.sync.dma_start(cc_in0[:].rearrange("o (p q) -> (o p) q", p=128), t_st[:])
        nc.gpsimd.collective_compute(
            "AllGather", mybir.AluOpType.bypass,
            replica_groups=[[0, 1], [2, 3], [4, 5], [6, 7]],
            ins=[cc_in0[:]], outs=[cc_out0[:]])
        # per-channel A1 scales
        t_scl = mpool.tile([128, 2], f32, tag="scl")
        nc.sync.dma_start(t_scl[:], a_F32[:, :256].rearrange("o (b p) -> (o p) b", p=128))

        # build padded canvases: u8 staging -> ACT copy*scale -> f16 interior
        apad_next = []
        cc0_f = cc_out0[:].rearrange("h (b c q) -> h b c q", b=2, c=128)
        for blk in range(2):
            t = apool.tile([128, NPIX_PAD], f16, tag="apad")
            nc.vector.memset(t[:], 0.0)
            t_s = q4pool.tile([128, NPIX], u8, tag="q4")
            for h in range(2):
                nc.sync.dma_start(t_s[:, h * PXH:(h + 1) * PXH], cc0_f[h, blk])
            t3 = t[:].rearrange("p (y x) -> p y x", y=HP)
            nc.scalar.activation(
                t3[:, PAD:PAD + H, PAD:PAD + W],
                t_s[:].rearrange("p (y x) -> p y x", y=H),
                mybir.ActivationFunctionType.Copy,
                scale=t_scl[:, blk:blk + 1])
            apad_next.append(t)

        for l in range(1, 8):
            nblk = _CIN[l] // 128
            apads = apad_next

            t_idx = idxpool.tile([128, IDX_COLS], i16, tag="idx")
            idx_src = a_I16[:, OFF_IDX // 2 + (l - 1) * 16 * IDX_COLS:
                            OFF_IDX // 2 + l * 16 * IDX_COLS].rearrange(
                "o (p q) -> (o p) q", p=16)
            for g in range(8):
                nc.sync.dma_start(t_idx[16 * g:16 * g + 16, :], idx_src)
            t_wt = wtpool.tile([128, nblk * NTAPS * 128], f16, tag="wt")
            if l == 1:
                wt_src = wt_all[0:2, :].rearrange("a (t p m) -> (a t) p m", p=128, m=128)
            else:
                wt_src = wt_all[l, :].rearrange("(t p m) -> t p m", p=128, m=128)
            nc.sync.dma_start(
                t_wt[:].rearrange("p (t m) -> p t m", m=128),
                wt_src.transpose([1, 0, 2]))
            t_bias = mpool.tile([128, 1], f32, tag="bias")
            nc.sync.dma_start(
                t_bias[:],
                a_F32[:, 256 + (l - 1) * 128:256 + l * 128].rearrange(
                    "o (p q) -> (o p) q", p=128))

            t_ps = pspool.tile([128, PXH], f32, tag="psacc")
            for blk in range(nblk):
                # Q4 pack: [128, q, dy, dx] <- A_pad[q + {0,1,WP,WP+1}]
                t_q4 = q4pool.tile([128, NPIX_PAD * 4], f16, tag="q4")
                src = apads[blk][:]
                src_view = bass.AP(
                    tensor=src.tensor, offset=src.offset,
                    ap=[list(src.ap[0]), [1, Q4_BUILD], [WP, 2], [1, 2]])
                dst = t_q4[:]
                dst_view = bass.AP(
                    tensor=dst.tensor, offset=dst.offset,
                    ap=[list(dst.ap[0]), [4, Q4_BUILD], [2, 2], [1, 2]])
                nc.vector.tensor_copy(dst_view, src_view)
                for chunk in range(3):
                    t_g = gpool.tile([128, NI_CHUNK * 4], f16, tag="g")
                    nc.gpsimd.ap_gather(
                        t_g[:], t_q4[:, :4 * Q4_BUILD],
                        t_idx[:, chunk * (NI_CHUNK // 16):(chunk + 1) * (NI_CHUNK // 16)],
                        channels=128, num_elems=Q4_BUILD, d=4, num_idxs=NI_CHUNK)
                    t_fu8 = mpool.tile([1, CHUNK_TAPS * FXY_PER_TAP], u8, tag="fu8")
                    fo = OFF_FXY + ((l - 1) * NTAPS + chunk * CHUNK_TAPS) * FXY_PER_TAP
                    nc.sync.dma_start(
                        t_fu8[:], a_U8[:, fo:fo + CHUNK_TAPS * FXY_PER_TAP])
                    for t in range(CHUNK_TAPS):
                        k = CHUNK_TAPS * chunk + t
                        t_wq = wqpool.tile([1, PXH * 4], f16, tag="wqr")
                        fxu8 = t_fu8[:, t * FXY_PER_TAP:t * FXY_PER_TAP + PXH]
                        fyu8 = t_fu8[:, t * FXY_PER_TAP + PXH:(t + 1) * FXY_PER_TAP]
                        w4v = t_wq[:].rearrange("o (q j) -> o q j", j=4)
                        # weight build in-place ([0]=w00 [1]=w01 [2]=w10 [3]=w11):
                        #   s1=fx, s2=fy, s3=fx*fy, s0=1-fx,
                        #   s2=fy*(1-fx), s1=fx-fx*fy, s0=(1-fx)-fy*(1-fx)
                        nc.vector.tensor_scalar(w4v[:, :, 1], fxu8, 1.0 / 255.0, None,
                                                op0=mybir.AluOpType.mult)
                        nc.vector.tensor_scalar(w4v[:, :, 2], fyu8, 1.0 / 255.0, None,
                                                op0=mybir.AluOpType.mult)
                        nc.vector.tensor_mul(w4v[:, :, 3], w4v[:, :, 2], w4v[:, :, 1])
                        nc.vector.tensor_scalar(w4v[:, :, 0], fxu8, -1.0 / 255.0, 1.0,
                                                op0=mybir.AluOpType.mult, op1=mybir.AluOpType.add)
                        nc.vector.tensor_mul(w4v[:, :, 2], w4v[:, :, 2], w4v[:, :, 0])
                        nc.vector.tensor_sub(w4v[:, :, 1], w4v[:, :, 1], w4v[:, :, 3])
                        nc.vector.tensor_sub(w4v[:, :, 0], w4v[:, :, 0], w4v[:, :, 2])
                        t_wb = wbpool.tile([128, PXH * 4], f16, tag="wb")
                        nc.gpsimd.partition_broadcast(t_wb[:], t_wq[:])
                        g_slice = t_g[:, t * PXH * 4:(t + 1) * PXH * 4]
                        nc.vector.tensor_mul(g_slice, g_slice, t_wb[:])
                        t_bk = bkpool.tile([128, PXH], f16, tag="bk")
                        with nc.allow_low_precision("f16 im2col"):
                            nc.vector.tensor_reduce(
                                t_bk[:],
                                g_slice.rearrange("p (q j) -> p q j", j=4),
                                axis=mybir.AxisListType.X, op=mybir.AluOpType.add)
                        lhsT = t_wt[:, (blk * NTAPS + k) * 128:(blk * NTAPS + k + 1) * 128]
                        first = (blk == 0 and k == 0)
                        last = (blk == nblk - 1 and k == NTAPS - 1)
                        for nck in range(4):
                            nc.tensor.matmul(
                                t_ps[:, nck * 512:(nck + 1) * 512],
                                lhsT, t_bk[:, nck * 512:(nck + 1) * 512],
                                start=first, stop=last)

            # eviction: relu(psum + bias)
            t_ev = evpool.tile([128, PXH], f16, tag="ev")
            nc.scalar.activation(t_ev[:], t_ps[:], mybir.ActivationFunctionType.Relu,
                                 bias=t_bias[:], scale=1.0)

            if l < 7:
                nc.sync.dma_start(
                    cc_in[l][:].rearrange("o (p q) -> (o p) q", p=128), t_ev[:])
                nc.gpsimd.collective_compute(
                    "AllGather", mybir.AluOpType.bypass,
                    replica_groups=[[0, 1], [2, 3], [4, 5], [6, 7]],
                    ins=[cc_in[l][:]], outs=[cc_out[l][:]])
                t_an = apool.tile([128, NPIX_PAD], f16, tag="apad")
                nc.vector.memset(t_an[:], 0.0)
                an3 = t_an[:].rearrange("p (y x) -> p y x", y=HP)
                cc3 = cc_out[l][:].rearrange("h (c y x) -> h c y x", c=128, y=H // 2)
                for h in range(2):
                    nc.sync.dma_start(
                        an3[:, PAD + 32 * h:PAD + 32 * h + 32, PAD:PAD + W],
                        cc3[h])
                apad_next = [t_an]
            else:
                # u8 quantize: per-channel max -> q = y * (254/max)
                t_mx = mpool.tile([128, 1], f32, tag="ymx")
                nc.vector.tensor_reduce(t_mx[:], t_ev[:],
                                        axis=mybir.AxisListType.X,
                                        op=mybir.AluOpType.max)
                nc.vector.tensor_scalar(t_mx[:], t_mx[:], 1e-12, None,
                                        op0=mybir.AluOpType.max)
                t_qs = mpool.tile([128, 1], f32, tag="yqs")
                nc.vector.reciprocal(t_qs[:], t_mx[:])
                nc.vector.tensor_scalar(t_qs[:], t_qs[:], 254.0, None,
                                        op0=mybir.AluOpType.mult)
                t_q = evpool.tile([128, PXH], u8, tag="yq")
                nc.scalar.activation(t_q[:], t_ev[:],
                                     mybir.ActivationFunctionType.Copy,
                                     scale=t_qs[:])
                nc.sync.dma_start(a_y[:, :PXH], t_q[:])
                nc.sync.dma_start(a_y[:, PXH:], t_mx[:].bitcast(mybir.dt.uint8))

    nc.compile()
    return nc


# ---------------- host input prep ----------------

def _prep_in_maps(inputs, A1):
    """Build the per-core consolidated u8 input blob."""
    in_maps = []
    N = A1.shape[0]
    # per-sample A1 u8 quantization (scales shared by the core pair)
    a1_q, a1_scale = [], []
    for s in range(N):
        mx = np.maximum(A1[s].max(axis=1), 1e-12).astype(np.float32)
        q = np.rint(A1[s] * (255.0 / mx)[:, None]).astype(np.uint8)
        a1_q.append(q)
        a1_scale.append((mx / 255.0).astype(np.float32))
    # weight shard source (same for all cores; shard by core id)
    const_parts = []
    for l in range(1, 8):
        wl = np.asarray(inputs[f"w{l}"], np.float32)
        nblk = _CIN[l] // 128
        wdt = np.float16 if COMPUTE_DT == "f16" else bf16
        wt = np.empty((nblk * NTAPS, 128, 128), wdt)
        for blk in range(nblk):
            for k in range(NTAPS):
                kh, kw = divmod(k, K)
                wt[blk * NTAPS + k] = wl[:, blk * 128:(blk + 1) * 128, kh, kw].T.astype(wdt)
        const_parts.append(wt.reshape(-1))
    wt_flat = np.concatenate(const_parts)
    biases = np.concatenate([np.asarray(inputs[f"b{l}"], np.float32) for l in range(1, 8)])

    for core in range(NCORES):
        s, h = core // 2, core % 2
        px_sel = slice(h * PXH, (h + 1) * PXH)
        fb = np.concatenate([a1_scale[s], biases]).astype('<f4')
        wtc = wt_flat[core * WT_CHUNK:(core + 1) * WT_CHUNK]
        idx_parts, fxy_parts = [], []
        for l in range(1, 8):
            q00, w4 = _precompute_layer(np.asarray(inputs[f"off{l}"][s], np.float32), 1)
            qh = q00[:, px_sel]                  # [9, 2048]
            wh = w4[:, px_sel, :]                # [9, 2048, 4]
            assert qh.max() < Q4_BUILD
            idx_chunks = [
                qh[c * CHUNK_TAPS:(c + 1) * CHUNK_TAPS].reshape(-1, 16).T.astype('<i2')
                for c in range(3)]
            idx_parts.append(np.concatenate(idx_chunks, axis=1))
            assert np.abs(wh.sum(-1) - 1.0).max() < 1e-5, "corner mask active; fx/fy form invalid"
            fxh = wh[:, :, 1] + wh[:, :, 3]      # [9, 2048]
            fyh = wh[:, :, 2] + wh[:, :, 3]
            fxy = np.stack([fxh, fyh], axis=1)   # [9, 2, 2048]
            fxy_parts.append(np.rint(fxy * 255.0).astype(np.uint8).reshape(-1))
        blob = np.concatenate([
            fb.view(np.uint8),
            wtc.view(np.uint8),
            np.stack(idx_parts).reshape(-1).view(np.uint8),
            a1_q[s][:, px_sel].reshape(-1),
            np.concatenate(fxy_parts),
        ])
        assert blob.nbytes == TOT_BYTES
        in_maps.append({"A": blob.reshape(1, -1)})
    return in_maps


def _decode_y(ybytes):
    """(128, 2052) u8 -> (128, 2048) f32."""
    q = ybytes[:, :PXH].astype(np.float32)
    mx = ybytes[:, PXH:].copy().view('<f4')      # [128, 1]
    return q * (mx / 254.0)


# ---------------- entry point ----------------

_LAST_RUN_NS = None
_NC_CACHE = None


def kernel(**inputs):
    global _LAST_RUN_NS, _NC_CACHE
    _t0 = _time.time()
    inputs = {k: np.asarray(v) for k, v in inputs.items()}
    x = inputs["x"].astype(np.float32)
    N = x.shape[0]
    assert N * 2 == NCORES

    # layer 0 on host
    A1 = np.stack([
        _host_l0(x[n], np.asarray(inputs["off0"][n], np.float32),
                 np.asarray(inputs["w0"], np.float32),
                 np.asarray(inputs["b0"], np.float32))
        for n in range(N)])                      # [N, 256, NPIX] f32

    _t1 = _time.time()
    if _NC_CACHE is None:
        _NC_CACHE = _build_program()
    nc = _NC_CACHE
    _t2 = _time.time()

    in_maps = _prep_in_maps(inputs, A1)

    _t3 = _time.time()
    res = bass_utils.run_bass_kernel_spmd(nc, in_maps, core_ids=list(range(NCORES)))
    _t4 = _time.time()
    _LAST_RUN_NS = int((_t4 - _t3) * 1e9)
    print(f"[kernel] host_l0={_t1-_t0:.2f}s build={_t2-_t1:.2f}s prep={_t3-_t2:.2f}s "
          f"run={_t4-_t3:.2f}s")

    out = np.empty((N, 128, H, W), np.float32)
    for core in range(NCORES):
        s, h = core // 2, core % 2
        y = _decode_y(res.results[core]["y"])    # [128, 2048] f32
        out[s, :, 32 * h:32 * h + 32, :] = y.reshape(128, 32, W)
    return out
